# revision 5
# baseline (speedup 1.0000x reference)
# BERT self-attention with relation bias (Tableformer) on 8 TRN2 NeuronCores.
#
# Strategy (per core = one batch element, pure data parallelism over B=8):
#   - Q^T/K^T/V projections in bf16 on TensorE (inputs pre-transposed host-side,
#     which is pure layout marshalling; all arithmetic runs on-device).
#   - scores computed TRANSPOSED: S^T[k, q] = sum_d K^T[d,k] * Q^T[d,q] so the
#     attention-mask add and softmax plumbing use per-partition (k) bias slots.
#   - softmax without max-subtraction (scores are O(1) here); the relation bias
#     is applied MULTIPLICATIVELY after exp:  exp(s + E[r,h]) = exp(s)*m_h[r].
#   - m_h[r] is normalized by m_h[6] (a per-head constant scale of the whole
#     softmax row cancels between numerator and denominator), leaving a 6-entry
#     table applied as TWO chained custom-DVE ops, each a fused "3-entry
#     lookup * multiply": entries 0/1 come from the per-partition scalar slots
#     s0/s1; entry 2 rides the instruction's imm2 immediate (the program is
#     (re)built per distinct rel_emb table, cached on those bytes).
#     Pass A reads the plane (rel-3): its <1 arm fires for rel<=3 so it applies
#     m3 to rel in {0,1,2,3} (and m4/m5 for rel 4/5, 1 for rel 6); pass B reads
#     raw rel with entries m0/m3, m1/m3, m2/m3 which cancels the spurious m3 on
#     rel in {0,1,2}. The product is exactly m_rel for every rel.
#   - ctx^T via a second matmul with P^T as the stationary operand; the softmax
#     denominator comes from a ones-column appended to V (column 64 of V').
#   - final division by the row-sum via ACT Identity with a per-partition
#     reciprocal scale; output assembled in SBUF and DMA'd out.
#   - for timing, the benchmark also builds an N-times unrolled copy of the
#     same program (the full body, input DMAs included, repeated N times in one
#     NEFF); (T_N - T_1)/(N-1) then measures pure on-device execution time of
#     one iteration -- host/tunnel dispatch overhead cancels exactly.
import os
import sys
import numpy as np

sys.path.insert(0, "/opt/trn_rl_repo")

import concourse.mybir as mybir  # noqa: E402
from concourse import bass, bacc, tile  # noqa: E402
from concourse.bass_utils import run_bass_kernel_spmd  # noqa: E402
from concourse.dve_ops import DveOp, OPS, CUSTOM_DVE_SPECS, get_dve_sub_opcode  # noqa: E402
from concourse.dve_spec import (  # noqa: E402
    Spec, Src0, Src1, C0, C1, C2, One, Zero, select, eq, lower, _has_src1, Bin,
)
from concourse.dve_uop import DveOpSpec, AluOp  # noqa: E402

B, S, D, H, HD, NREL = 8, 1024, 1024, 16, 64, 7
_ABLATE = os.environ.get("KERNEL_ABLATE", "none")  # timing experiments only
N_CORES = 8
P = 128
NT = S // P  # 8 tiles along any 1024 dim
F32 = mybir.dt.float32
BF16 = mybir.dt.bfloat16
I32 = mybir.dt.int32
I8 = mybir.dt.int8
AF = mybir.ActivationFunctionType
OP = mybir.AluOpType

# ---------------------------------------------------------------------------
# Custom DVE op: out = (in0<1 ? s0 : in0==1 ? s1 : in0==2 ? imm2 : 1) * in1
# (3-entry lookup-multiply; entry 0 fires for any in0 < 1 incl. negatives)
# ---------------------------------------------------------------------------
_LUT3 = None


def _register_lut3():
    global _LUT3
    if _LUT3 is not None:
        return _LUT3
    for op in OPS:
        if op.name == "REL_LUT3_MUL":
            _LUT3 = op
            return op
    lt = lambda a, b: Bin(AluOp.IS_LT, a, b)  # noqa: E731
    sub = lambda a, b: Bin(AluOp.SUBTRACT, a, b)  # noqa: E731
    body = select(lt(Src0, One), C0,
           select(eq(Src0, One), C1,
           select(eq(sub(Src0, One), One), C2, One))) * Src1

    def _ref(in0, in1, s0, s1, imm2):
        return (
            np.where(in0 < 1, s0, np.where(in0 == 1, s1,
            np.where(in0 == 2, np.float32(imm2), np.float32(1.0)))) * in1
        )

    spec = Spec(body=body, reference=_ref)
    import concourse.dve_ops as _dvo
    op = DveOp("REL_LUT3_MUL", spec, subdim=False, uops_sha={})
    OPS.append(op)
    CUSTOM_DVE_SPECS[op.name] = spec
    _dvo._SUB_OPCODE_FOR_NAME[op.name] = _dvo._CUSTOM_DVE_ROW_BASE + len(OPS) - 1
    assert _dvo._SUB_OPCODE_FOR_NAME[op.name] < 0x20
    # pin the golden hashes dynamically (what DveOp.compile checks)
    for ver in ("v3", "v4"):
        try:
            d = DveOpSpec(
                name=op.name,
                opcode=get_dve_sub_opcode(op.name),
                uops=lower(spec, ver=ver),
                rd1_en=_has_src1(spec),
            )
            op.uops_sha[ver] = d.sha(ver)
        except Exception:
            pass
    _LUT3 = op
    return op


# ---------------------------------------------------------------------------
# Program builder. imm_tab bakes the per-head imm2 lookup entries (the m5 and
# m2/m3 rows of the normalized multiplier tables); the program cache in
# _get_program is keyed on those bytes so a different rel_emb rebuilds.
#
# The projection work is interleaved into the attention head loop so the
# Tensor engine's projection matmuls fill the slack while the (bottleneck)
# Vector engine chews the relation-bias ladder: only K/Q block 0 is projected
# up front; V' halves and K/Q blocks 1..7 are emitted between heads.
# ---------------------------------------------------------------------------
def _build_program(imm_tab, unroll=1):
    m5s, m2ns = imm_tab  # tuples of H floats each
    lut3 = _register_lut3()

    nc = bacc.Bacc(
        "TRN2",
        target_bir_lowering=False,
        debug=False,
        enable_asserts=False,
        num_devices=N_CORES,
    )

    # DRAM I/O (per core)
    xT_d = nc.dram_tensor("xT", [D, S], BF16, kind="ExternalInput")      # hidden[b].T  [din, seq]
    wqT_d = nc.dram_tensor("wqT", [D, D], BF16, kind="ExternalInput")    # Wq.T [din, dout]
    wkT_d = nc.dram_tensor("wkT", [D, D], BF16, kind="ExternalInput")
    wvT_d = nc.dram_tensor("wvT", [D, D], BF16, kind="ExternalInput")
    bq_d = nc.dram_tensor("bq", [D], F32, kind="ExternalInput")
    bk_d = nc.dram_tensor("bk", [D], F32, kind="ExternalInput")
    bv_d = nc.dram_tensor("bv", [D], F32, kind="ExternalInput")
    relT_d = nc.dram_tensor("relT", [S, S], I8, kind="ExternalInput")    # relation[b].T  [k, q]
    mask_d = nc.dram_tensor("maskv", [S], F32, kind="ExternalInput")     # attention_mask[b,0,0,:]
    mtab_d = nc.dram_tensor("mtab", [6 * H], F32, kind="ExternalInput")  # m'_rh host-computed
    out_d = nc.dram_tensor("out", [S, D], F32, kind="ExternalOutput")

    from contextlib import ExitStack

    with tile.TileContext(nc) as tc:
      for _it in range(unroll):
        with ExitStack() as ctx:
            const = ctx.enter_context(tc.tile_pool(name="const", bufs=1))

            # persistent SBUF tensors (live through attention phase)
            qT = const.tile([P, NT * S], BF16)       # Q^T/8 (+bq/8), dout on partitions
            kT = const.tile([P, NT * S], BF16)       # K^T  (+bk)
            vP = const.tile([P, NT * H * (HD + 1)], BF16)  # V' per seq-block: 16*(64+1)
            rel0 = const.tile([P, NT * S], BF16)     # rel^T as bf16
            relm3 = const.tile([P, NT * S], BF16)    # rel^T - 3
            out_sb = const.tile([P, NT * S], F32)    # output rows, q on partitions
            mcols = const.tile([P, NT], F32)         # mask column per k-tile
            bqcols = const.tile([P, NT], F32)        # bq/8 column per dout-block
            bkcols = const.tile([P, NT], F32)
            mprime = const.tile([P, 6 * H], F32)     # m'_rh broadcast to partitions
            ones_row = const.tile([1, P], F32)       # lhsT for broadcast matmul
            ones_row_bf = const.tile([1, P], BF16)   # bf16 lhsT for rank-1 bias matmul
            bv_row2 = const.tile([1, D], BF16)       # bv as a single-partition row

            xpool = ctx.enter_context(tc.tile_pool(name="xpool", bufs=1))
            wkqp = ctx.enter_context(tc.tile_pool(name="wkq", bufs=4))
            wvp = ctx.enter_context(tc.tile_pool(name="wv", bufs=1))
            proj_ps = ctx.enter_context(tc.tile_pool(name="proj_ps", bufs=2, space="PSUM"))
            ptp = ctx.enter_context(tc.tile_pool(name="pt", bufs=2))
            sc_psp = ctx.enter_context(tc.tile_pool(name="sc_ps", bufs=2, space="PSUM"))
            cx_psp = ctx.enter_context(tc.tile_pool(name="cx_ps", bufs=2, space="PSUM"))
            exp_pool = ctx.enter_context(tc.tile_pool(name="ex", bufs=2))
            lad = ctx.enter_context(tc.tile_pool(name="lad", bufs=1))
            rcp = ctx.enter_context(tc.tile_pool(name="rc", bufs=2))

            xT = xpool.tile([P, NT * S], BF16)

            # ---------------- prefix: input loads + constants ----------------
            # Single strided DMAs (one DMACopy each; rings still parallelize):
            # X^T gates the first projection, rel planes gate the first ladder.
            for t in range(NT):
                nc.gpsimd.dma_start(
                    out=xT[:, t * S:(t + 1) * S], in_=xT_d[t * P:(t + 1) * P, :]
                )
            for t in range(NT):
                # DMA-cast int8 -> bf16; values 0..6 exact
                nc.gpsimd.dma_start(
                    out=rel0[:, t * S:(t + 1) * S], in_=relT_d[t * P:(t + 1) * P, :]
                )
                # relm3 = rel - 3 on the (otherwise idle) Pool engine
                nc.gpsimd.tensor_scalar(
                    relm3[:, t * S:(t + 1) * S], rel0[:, t * S:(t + 1) * S],
                    -3.0, None, OP.add,
                )

            with tc.tile_pool(name="prep", bufs=1) as prep:
                # mask / bias columns: v[t*128+p] -> [p, t]
                nc.sync.dma_start(out=mcols[:], in_=mask_d[:].rearrange("(t p) -> p t", p=P))
                nc.sync.dma_start(out=bqcols[:], in_=bq_d[:].rearrange("(t p) -> p t", p=P))
                nc.sync.dma_start(out=bkcols[:], in_=bk_d[:].rearrange("(t p) -> p t", p=P))
                nc.vector.tensor_scalar_mul(bqcols[:], bqcols[:], 0.125)

                nc.vector.memset(ones_row[:], 1.0)
                nc.vector.memset(ones_row_bf[:], 1.0)

                # m' table broadcast to all partitions: [1,96] -> psum [128,96]
                mrow = prep.tile([1, 6 * H], F32)
                nc.sync.dma_start(
                    out=mrow[:], in_=mtab_d[:].rearrange("(o n) -> o n", o=1)
                )
                # borrow a scores-psum slot (pool is otherwise idle this early)
                mb_ps = sc_psp.tile([P, S], F32, tag="scps")
                nc.tensor.matmul(mb_ps[:, 0:6 * H], ones_row[:], mrow[:])
                nc.vector.tensor_copy(mprime[:], mb_ps[:, 0:6 * H])

                nc.gpsimd.dma_start(out=bv_row2[:], in_=bv_d[:].rearrange("(o d) -> o d", o=1))

            # V' gets ones in column 64 of each head slot
            nc.gpsimd.memset(vP[:], 1.0)

            # ---------------- projection emitters ----------------
            def emit_kq_block(which, c):
                """Project dout block c (heads 2c, 2c+1) of K or Q."""
                wsrc = wkT_d if which == "k" else wqT_d
                w = wkqp.tile([P, NT * P], BF16, tag="wblk")
                for t in range(NT):
                    nc.sync.dma_start(
                        out=w[:, t * P:(t + 1) * P],
                        in_=wsrc[t * P:(t + 1) * P, c * P:(c + 1) * P],
                    )
                dst = kT if which == "k" else qT
                bias_cols = bkcols if which == "k" else bqcols
                scale = 1.0 if which == "k" else 0.125
                for j in range(2):
                    ps = proj_ps.tile([P, 512], F32, tag="pps")
                    for kk in range(NT):
                        nc.tensor.matmul(
                            ps[:],
                            w[:, kk * P:(kk + 1) * P],
                            xT[:, kk * S + j * 512: kk * S + (j + 1) * 512],
                            start=(kk == 0),
                            stop=(kk == NT - 1),
                        )
                    nc.scalar.activation(
                        dst[:, c * S + j * 512: c * S + (j + 1) * 512], ps[:],
                        AF.Identity, bias=bias_cols[:, c:c + 1], scale=scale,
                    )

            def emit_v_half(half):
                """Project V' for heads half*8 .. half*8+7 (dout cols 512*half..)."""
                wv = wvp.tile([P, NT * 512], BF16, tag="wv")
                for t in range(NT):
                    nc.sync.dma_start(
                        out=wv[:, t * 512:(t + 1) * 512],
                        in_=wvT_d[t * P:(t + 1) * P, half * 512:(half + 1) * 512],
                    )
                for sb in range(NT):
                    ps = proj_ps.tile([P, 512], F32, tag="pps")
                    for kk in range(NT):
                        nc.tensor.matmul(
                            ps[:],
                            xT[:, kk * S + sb * P: kk * S + (sb + 1) * P],
                            wv[:, kk * 512:(kk + 1) * 512],
                            start=(kk == 0),
                            stop=False,
                        )
                    # + bv via a rank-1 accumulating matmul (ones column x bv row)
                    nc.tensor.matmul(
                        ps[:],
                        ones_row_bf[:],
                        bv_row2[:, half * 512:(half + 1) * 512],
                        start=False,
                        stop=True,
                    )
                    vslot = vP[
                        :, sb * H * 65 + half * 8 * 65: sb * H * 65 + (half * 8 + 8) * 65
                    ].rearrange("p (h e) -> p h e", h=8)[:, :, 0:HD]
                    nc.scalar.activation(
                        vslot, ps[:].rearrange("p (h e) -> p h e", h=8), AF.Copy,
                    )

            # ---------------- attention (with interleaved projections) --------
            def emit_ctx(h, pt, last=False):
                for qb in range(NT):
                    cps = cx_psp.tile([P, HD + 1], F32, tag="cps")
                    for kb in range(NT):
                        nc.tensor.matmul(
                            cps[:],
                            pt[:, kb * S + qb * P: kb * S + (qb + 1) * P],
                            vP[:, kb * H * 65 + h * 65: kb * H * 65 + (h + 1) * 65],
                            start=(kb == 0),
                            stop=(kb == NT - 1),
                        )
                    rc = rcp.tile([P, 1], F32, tag="rc")
                    nc.vector.reciprocal(rc[:], cps[:, HD:HD + 1])
                    nc.scalar.activation(
                        out_sb[:, qb * S + h * HD: qb * S + (h + 1) * HD],
                        cps[:, 0:HD], AF.Identity, bias=0.0, scale=rc[:],
                    )
                    if last:
                        nc.sync.dma_start(
                            out=out_d[qb * P:(qb + 1) * P, :],
                            in_=out_sb[:, qb * S:(qb + 1) * S],
                        )

            emit_kq_block("k", 0)
            emit_kq_block("q", 0)

            prev = None
            for h in range(H):
                off = (h % 2) * HD
                hc = h // 2
                pt = ptp.tile([P, NT * S], BF16, tag="pt")
                for kb2 in range(NT // 4):
                    # four k-tiles share one exp buffer so the custom-DVE ladder
                    # runs at FD=4096, amortizing per-op drain/dispatch overhead
                    ex = exp_pool.tile([P, 4 * S], BF16, tag="ex")
                    for kh in range(4):
                        kb = kb2 * 4 + kh
                        ps = sc_psp.tile([P, S], F32, tag="scps")
                        for j in range(2):
                            nc.tensor.matmul(
                                ps[:, j * 512:(j + 1) * 512],
                                kT[off:off + HD, hc * S + kb * P: hc * S + (kb + 1) * P],
                                qT[off:off + HD, hc * S + j * 512: hc * S + (j + 1) * 512],
                            )
                        nc.scalar.activation(
                            ex[:, kh * S:(kh + 1) * S], ps[:], AF.Exp,
                            bias=mcols[:, kb:kb + 1], scale=1.0,
                        )
                    kb = kb2 * 4
                    ptk = pt[:, kb * S:(kb + 4) * S]
                    t1 = lad.tile([P, 4 * S], BF16, tag="l1")
                    if _ABLATE == "ts":
                        # timing ablation: same op count/shapes, native 4x ops
                        nc.vector.tensor_scalar(
                            t1[:], ex[:], 1.0,
                            mprime[:, 0 * H + h: 0 * H + h + 1], OP.mult, OP.mult,
                        )
                        nc.vector.tensor_scalar(
                            ptk, t1[:], 1.0,
                            mprime[:, 3 * H + h: 3 * H + h + 1], OP.mult, OP.mult,
                        )
                    else:
                        # pass A on (rel-3): m3 for rel<=3, m4/m5 for rel 4/5
                        nc.vector._custom_dve(
                            lut3, out=t1[:], in0=relm3[:, kb * S:(kb + 4) * S], in1=ex[:],
                            s0=mprime[:, 3 * H + h: 3 * H + h + 1],
                            s1=mprime[:, 4 * H + h: 4 * H + h + 1],
                            imm2=m5s[h],
                        )
                        # pass B on raw rel: m0/m3, m1/m3, m2/m3 for rel 0/1/2
                        nc.vector._custom_dve(
                            lut3, out=ptk, in0=rel0[:, kb * S:(kb + 4) * S], in1=t1[:],
                            s0=mprime[:, 0 * H + h: 0 * H + h + 1],
                            s1=mprime[:, 1 * H + h: 1 * H + h + 1],
                            imm2=m2ns[h],
                        )

                # Interleave projection chunks so the Tensor engine always has
                # just-in-time work, spread as thin as dependencies allow:
                # V'(heads 0-7) after head 0 (first needed by ctx(0) at head 1);
                # V' half 1 at head 8 (needed by ctx(8) at head 9); K/Q block c
                # at head 2c-2, two heads before scores(2c) consume it. ctx runs
                # one head behind.
                if h == 0:
                    emit_v_half(0)
                else:
                    if h == 1:
                        emit_kq_block("k", 1)
                        emit_kq_block("q", 1)
                    elif h % 2 == 0 and 2 <= h <= 12:
                        emit_kq_block("k", h // 2 + 1)
                        emit_kq_block("q", h // 2 + 1)
                    if h == 8:
                        emit_v_half(1)
                    emit_ctx(*prev)
                prev = (h, pt)

            emit_ctx(*prev, last=True)

    nc.compile()
    return nc


_PROGRAMS = {}


def _mtables(inputs):
    """Normalized multiplier table, host-side (O(NREL*H)=112 values of table
    prep, not per-element work). Rows 0..2 hold m'_r/m'_3 (pass-B entries),
    rows 3..5 hold m'_r (pass-A entries), with m'_r = exp(E[r]-E[6])."""
    remb = np.asarray(inputs["rel_emb"], dtype=np.float32)
    mp = np.exp(remb[0:6, :] - remb[6:7, :]).astype(np.float32)  # [6, H]
    out = mp.copy()
    out[0:3, :] = mp[0:3, :] / mp[3:4, :]
    return out


def _get_program(inputs, unroll=1):
    mt = _mtables(inputs)
    m5s = tuple(float(x) for x in mt[5])
    m2ns = tuple(float(x) for x in mt[2])
    key = (m5s, m2ns, _ABLATE, unroll)
    prog = _PROGRAMS.get(key)
    if prog is None:
        prog = _build_program((m5s, m2ns), unroll=unroll)
        _PROGRAMS[key] = prog
    return prog


def _make_in_maps(inputs):
    import ml_dtypes
    bf16 = ml_dtypes.bfloat16
    hidden = np.asarray(inputs["hidden_states"], dtype=np.float32)
    mask = np.asarray(inputs["attention_mask"], dtype=np.float32)
    relation = np.asarray(inputs["relation"], dtype=np.int32)
    wq = np.ascontiguousarray(np.asarray(inputs["Wq"], dtype=np.float32).T.astype(bf16))
    wk = np.ascontiguousarray(np.asarray(inputs["Wk"], dtype=np.float32).T.astype(bf16))
    wv = np.ascontiguousarray(np.asarray(inputs["Wv"], dtype=np.float32).T.astype(bf16))
    bq = np.asarray(inputs["bq"], dtype=np.float32)
    bk = np.asarray(inputs["bk"], dtype=np.float32)
    bv = np.asarray(inputs["bv"], dtype=np.float32)
    mtab = np.ascontiguousarray(_mtables(inputs).reshape(-1))  # [6*H]

    in_maps = []
    for b in range(N_CORES):
        in_maps.append({
            "xT": np.ascontiguousarray(hidden[b].T.astype(bf16)),
            "wqT": wq, "wkT": wk, "wvT": wv,
            "bq": bq, "bk": bk, "bv": bv,
            "relT": np.ascontiguousarray(relation[b].T.astype(np.int8)),
            "maskv": np.ascontiguousarray(mask[b, 0, 0, :]),
            "mtab": mtab,
        })
    return in_maps


LAST_EXEC_NS = None
LAST_RESULTS = None


def kernel(**inputs) -> np.ndarray:
    global LAST_EXEC_NS, LAST_RESULTS
    nc = _get_program(inputs)
    in_maps = _make_in_maps(inputs)
    trace = os.environ.get("KERNEL_TRACE", "0") == "1"
    res = run_bass_kernel_spmd(nc, in_maps, list(range(N_CORES)), trace=trace)
    LAST_EXEC_NS = res.exec_time_ns
    LAST_RESULTS = res
    out = np.stack([res.results[b]["out"] for b in range(N_CORES)], axis=0)
    return out.astype(np.float32)


# -------- timing helper: device-resident repeated dispatch --------
def make_bench_fn(inputs, unroll=1):
    """Returns run(reps) -> min wall seconds over reps dispatches of the
    unroll-times-unrolled program (device-resident inputs)."""
    import jax
    from jax.sharding import Mesh, PartitionSpec, NamedSharding
    from jax.experimental.shard_map import shard_map
    from concourse import bass2jax
    import concourse.mybir as mb

    nc = _get_program(inputs, unroll=unroll)
    in_maps = _make_in_maps(inputs)
    bass2jax.install_neuronx_cc_hook()

    part_name = nc.partition_id_tensor.name if nc.partition_id_tensor else None
    in_names, out_names, out_avals, zero_outs = [], [], [], []
    for alloc in nc.m.functions[0].allocations:
        if not isinstance(alloc, mb.MemoryLocationSet):
            continue
        name = alloc.memorylocations[0].name
        if alloc.kind == "ExternalInput":
            if name != part_name:
                in_names.append(name)
        elif alloc.kind == "ExternalOutput":
            out_names.append(name)
            shape = tuple(alloc.tensor_shape)
            dtype = mb.dt.np(alloc.dtype)
            out_avals.append(jax.core.ShapedArray(shape, dtype))
            zero_outs.append(np.zeros(shape, dtype))
    n_params = len(in_names)
    all_names = in_names + out_names
    if part_name is not None:
        all_names.append(part_name)

    def _body(*args):
        operands = list(args)
        if part_name is not None:
            operands.append(bass2jax.partition_id_tensor())
        outs = bass2jax._bass_exec_p.bind(
            *operands,
            out_avals=tuple(out_avals),
            in_names=tuple(all_names),
            out_names=tuple(out_names),
            lowering_input_output_aliases=(),
            sim_require_finite=True,
            sim_require_nnan=True,
            nc=nc,
        )
        return tuple(outs)

    devices = jax.devices()[:N_CORES]
    mesh = Mesh(np.asarray(devices), ("core",))
    n_all = n_params + len(out_names)
    sharded = jax.jit(
        shard_map(
            _body, mesh=mesh,
            in_specs=(PartitionSpec("core"),) * n_all,
            out_specs=(PartitionSpec("core"),) * len(out_names),
            check_rep=False,
        ),
        keep_unused=True,
    )
    sh = NamedSharding(mesh, PartitionSpec("core"))
    concat_in = [
        jax.device_put(
            np.concatenate([np.asarray(in_maps[c][nm]) for c in range(N_CORES)], axis=0), sh
        )
        for nm in in_names
    ]
    concat_zeros = [
        jax.device_put(np.zeros((N_CORES * z.shape[0], *z.shape[1:]), z.dtype), sh)
        for z in zero_outs
    ]

    # warmup + compile
    out = sharded(*concat_in, *concat_zeros)
    jax.block_until_ready(out)

    import time

    def run(reps=1):
        best = float("inf")
        for _ in range(reps):
            t0 = time.perf_counter()
            outs = sharded(*concat_in, *concat_zeros)
            jax.block_until_ready(outs)
            best = min(best, time.perf_counter() - t0)
        return best

    def get_out():
        outs = sharded(*concat_in, *concat_zeros)
        o = np.asarray(outs[0]).reshape(N_CORES, *out_avals[0].shape)
        return o

    run.get_out = get_out
    return run


# -------- simulation helper (single core) for test.py --------
def run_sim_core0(inputs):
    from concourse.bass_interp import CoreSim

    nc = _get_program(inputs)
    in_maps = _make_in_maps(inputs)
    sim = CoreSim(nc, trace=False)
    for k, v in in_maps[0].items():
        sim.tensor(k)[:] = v
    sim.simulate(check_with_hw=False)
    return np.array(sim.tensor("out"))


# revision 7
# speedup vs baseline: 1.1779x; 1.1779x over previous
# BERT self-attention with relation bias (Tableformer) on 8 TRN2 NeuronCores.
#
# Strategy (per core = one batch element, pure data parallelism over B=8):
#   - Q^T/K^T/V projections in bf16 on TensorE (inputs pre-transposed host-side,
#     which is pure layout marshalling; all arithmetic runs on-device).
#   - scores computed TRANSPOSED: S^T[k, q] = sum_d K^T[d,k] * Q^T[d,q] so the
#     attention-mask add and softmax plumbing use per-partition (k) bias slots.
#   - softmax without max-subtraction (scores are O(1) here); the relation bias
#     is applied MULTIPLICATIVELY after exp:  exp(s + E[r,h]) = exp(s)*m_h[r].
#   - m_h[r] is normalized by m_h[6] (a per-head constant scale of the whole
#     softmax row cancels between numerator and denominator), leaving a 6-entry
#     table applied as TWO chained custom-DVE ops, each a fused "3-entry
#     lookup * multiply": entries 0/1 come from the per-partition scalar slots
#     s0/s1; entry 2 rides the instruction's imm2 immediate (the program is
#     (re)built per distinct rel_emb table, cached on those bytes).
#     Pass A reads the plane (rel-3): its <1 arm fires for rel<=3 so it applies
#     m3 to rel in {0,1,2,3} (and m4/m5 for rel 4/5, 1 for rel 6); pass B reads
#     raw rel with entries m0/m3, m1/m3, m2/m3 which cancels the spurious m3 on
#     rel in {0,1,2}. The product is exactly m_rel for every rel.
#   - ctx^T via a second matmul with P^T as the stationary operand; the softmax
#     denominator comes from a ones-column appended to V (column 64 of V').
#   - final division by the row-sum via ACT Identity with a per-partition
#     reciprocal scale; output assembled in SBUF and DMA'd out.
#   - for timing, the benchmark also builds an N-times unrolled copy of the
#     same program (the full body, input DMAs included, repeated N times in one
#     NEFF); (T_N - T_1)/(N-1) then measures pure on-device execution time of
#     one iteration -- host/tunnel dispatch overhead cancels exactly.
import os
import sys
import numpy as np

sys.path.insert(0, "/opt/trn_rl_repo")

import concourse.mybir as mybir  # noqa: E402
from concourse import bass, bacc, tile  # noqa: E402
from concourse.bass_utils import run_bass_kernel_spmd  # noqa: E402
from concourse.dve_ops import DveOp, OPS, CUSTOM_DVE_SPECS, get_dve_sub_opcode  # noqa: E402
from concourse.dve_spec import (  # noqa: E402
    Spec, Src0, Src1, C0, C1, C2, One, Zero, select, eq, lower, _has_src1, Bin,
)
from concourse.dve_uop import DveOpSpec, AluOp  # noqa: E402

B, S, D, H, HD, NREL = 8, 1024, 1024, 16, 64, 7
_ABLATE = os.environ.get("KERNEL_ABLATE", "none")  # timing experiments only
N_CORES = 8
P = 128
NT = S // P  # 8 tiles along any 1024 dim
F32 = mybir.dt.float32
BF16 = mybir.dt.bfloat16
I32 = mybir.dt.int32
I8 = mybir.dt.int8
AF = mybir.ActivationFunctionType
OP = mybir.AluOpType

# ---------------------------------------------------------------------------
# Custom DVE op: out = (in0<1 ? s0 : in0==1 ? s1 : in0==2 ? imm2 : 1) * in1
# (3-entry lookup-multiply; entry 0 fires for any in0 < 1 incl. negatives)
# ---------------------------------------------------------------------------
_LUT3 = None


def _register_lut3():
    global _LUT3
    if _LUT3 is not None:
        return _LUT3
    for op in OPS:
        if op.name == "REL_LUT3_MUL":
            _LUT3 = op
            return op
    lt = lambda a, b: Bin(AluOp.IS_LT, a, b)  # noqa: E731
    sub = lambda a, b: Bin(AluOp.SUBTRACT, a, b)  # noqa: E731
    body = select(lt(Src0, One), C0,
           select(eq(Src0, One), C1,
           select(eq(sub(Src0, One), One), C2, One))) * Src1

    def _ref(in0, in1, s0, s1, imm2):
        return (
            np.where(in0 < 1, s0, np.where(in0 == 1, s1,
            np.where(in0 == 2, np.float32(imm2), np.float32(1.0)))) * in1
        )

    spec = Spec(body=body, reference=_ref)
    import concourse.dve_ops as _dvo
    op = DveOp("REL_LUT3_MUL", spec, subdim=False, uops_sha={})
    OPS.append(op)
    CUSTOM_DVE_SPECS[op.name] = spec
    _dvo._SUB_OPCODE_FOR_NAME[op.name] = _dvo._CUSTOM_DVE_ROW_BASE + len(OPS) - 1
    assert _dvo._SUB_OPCODE_FOR_NAME[op.name] < 0x20
    # pin the golden hashes dynamically (what DveOp.compile checks)
    for ver in ("v3", "v4"):
        try:
            d = DveOpSpec(
                name=op.name,
                opcode=get_dve_sub_opcode(op.name),
                uops=lower(spec, ver=ver),
                rd1_en=_has_src1(spec),
            )
            op.uops_sha[ver] = d.sha(ver)
        except Exception:
            pass
    _LUT3 = op
    return op


# ---------------------------------------------------------------------------
# Program builder. imm_tab bakes the per-head imm2 lookup entries (the m5 and
# m2/m3 rows of the normalized multiplier tables); the program cache in
# _get_program is keyed on those bytes so a different rel_emb rebuilds.
#
# The projection work is interleaved into the attention head loop so the
# Tensor engine's projection matmuls fill the slack while the (bottleneck)
# Vector engine chews the relation-bias ladder: only K/Q block 0 is projected
# up front; V' halves and K/Q blocks 1..7 are emitted between heads.
# ---------------------------------------------------------------------------
def _build_program(imm_tab, unroll=1):
    m5s, m2ns = imm_tab  # tuples of H floats each
    lut3 = _register_lut3()

    nc = bacc.Bacc(
        "TRN2",
        target_bir_lowering=False,
        debug=False,
        enable_asserts=False,
        num_devices=N_CORES,
    )

    # DRAM I/O (per core)
    xT_d = nc.dram_tensor("xT", [D, S], BF16, kind="ExternalInput")      # hidden[b].T  [din, seq]
    wqT_d = nc.dram_tensor("wqT", [D, D], BF16, kind="ExternalInput")    # Wq.T [din, dout]
    wkT_d = nc.dram_tensor("wkT", [D, D], BF16, kind="ExternalInput")
    wvT_d = nc.dram_tensor("wvT", [D, D], BF16, kind="ExternalInput")
    bq_d = nc.dram_tensor("bq", [D], F32, kind="ExternalInput")
    bk_d = nc.dram_tensor("bk", [D], F32, kind="ExternalInput")
    bv_d = nc.dram_tensor("bv", [D], F32, kind="ExternalInput")
    relT_d = nc.dram_tensor("relT", [S, S], I8, kind="ExternalInput")    # relation[b].T  [k, q]
    mask_d = nc.dram_tensor("maskv", [S], F32, kind="ExternalInput")     # attention_mask[b,0,0,:]
    mtab_d = nc.dram_tensor("mtab", [6 * H], F32, kind="ExternalInput")  # m'_rh host-computed
    out_d = nc.dram_tensor("out", [S, D], F32, kind="ExternalOutput")

    from contextlib import ExitStack

    with tile.TileContext(nc) as tc:
      for _it in range(unroll):
        with ExitStack() as ctx:
            const = ctx.enter_context(tc.tile_pool(name="const", bufs=1))

            # persistent SBUF tensors (live through attention phase)
            qT = const.tile([P, NT * S], BF16)       # Q^T/8 (+bq/8), dout on partitions
            kT = const.tile([P, NT * S], BF16)       # K^T  (+bk)
            vP = const.tile([P, NT * H * (HD + 1)], BF16)  # V' per seq-block: 16*(64+1)
            rel0 = const.tile([P, NT * S], BF16)     # rel^T as bf16
            relm3 = const.tile([P, NT * S], BF16)    # rel^T - 3
            out_sb = const.tile([P, NT * S], F32)    # output rows, q on partitions
            mcols = const.tile([P, NT], F32)         # mask column per k-tile
            bqcols = const.tile([P, NT], F32)        # bq/8 column per dout-block
            bkcols = const.tile([P, NT], F32)
            mprime = const.tile([P, 6 * H], F32)     # m'_rh broadcast to partitions
            ones_row = const.tile([1, P], F32)       # lhsT for broadcast matmul
            ones_row_bf = const.tile([1, P], BF16)   # bf16 lhsT for rank-1 bias matmul
            bv_row2 = const.tile([1, D], BF16)       # bv as a single-partition row

            xpool = ctx.enter_context(tc.tile_pool(name="xpool", bufs=1))
            wkqp = ctx.enter_context(tc.tile_pool(name="wkq", bufs=4))
            wvp = ctx.enter_context(tc.tile_pool(name="wv", bufs=1))
            proj_ps = ctx.enter_context(tc.tile_pool(name="proj_ps", bufs=2, space="PSUM"))
            ptp = ctx.enter_context(tc.tile_pool(name="pt", bufs=2))
            sc_psp = ctx.enter_context(tc.tile_pool(name="sc_ps", bufs=2, space="PSUM"))
            cx_psp = ctx.enter_context(tc.tile_pool(name="cx_ps", bufs=2, space="PSUM"))
            exp_pool = ctx.enter_context(tc.tile_pool(name="ex", bufs=2))
            lad = ctx.enter_context(tc.tile_pool(name="lad", bufs=1))
            rcp = ctx.enter_context(tc.tile_pool(name="rc", bufs=2))

            xT = xpool.tile([P, NT * S], BF16)

            # ---------------- prefix: input loads + constants ----------------
            # Single strided DMAs (one DMACopy each; rings still parallelize):
            # X^T gates the first projection, rel planes gate the first ladder.
            for t in range(NT):
                nc.gpsimd.dma_start(
                    out=xT[:, t * S:(t + 1) * S], in_=xT_d[t * P:(t + 1) * P, :]
                )
            for t in range(NT):
                # DMA-cast int8 -> bf16; values 0..6 exact
                nc.gpsimd.dma_start(
                    out=rel0[:, t * S:(t + 1) * S], in_=relT_d[t * P:(t + 1) * P, :]
                )
                # relm3 = rel - 3 on the (otherwise idle) Pool engine
                nc.gpsimd.tensor_scalar(
                    relm3[:, t * S:(t + 1) * S], rel0[:, t * S:(t + 1) * S],
                    -3.0, None, OP.add,
                )

            with tc.tile_pool(name="prep", bufs=1) as prep:
                # mask / bias columns: v[t*128+p] -> [p, t]
                nc.sync.dma_start(out=mcols[:], in_=mask_d[:].rearrange("(t p) -> p t", p=P))
                nc.sync.dma_start(out=bqcols[:], in_=bq_d[:].rearrange("(t p) -> p t", p=P))
                nc.sync.dma_start(out=bkcols[:], in_=bk_d[:].rearrange("(t p) -> p t", p=P))
                nc.vector.tensor_scalar_mul(bqcols[:], bqcols[:], 0.125)

                nc.vector.memset(ones_row[:], 1.0)
                nc.vector.memset(ones_row_bf[:], 1.0)

                # m' table broadcast to all partitions: [1,96] -> psum [128,96]
                mrow = prep.tile([1, 6 * H], F32)
                nc.sync.dma_start(
                    out=mrow[:], in_=mtab_d[:].rearrange("(o n) -> o n", o=1)
                )
                # borrow a scores-psum slot (pool is otherwise idle this early)
                mb_ps = sc_psp.tile([P, S], F32, tag="scps")
                nc.tensor.matmul(mb_ps[:, 0:6 * H], ones_row[:], mrow[:])
                nc.vector.tensor_copy(mprime[:], mb_ps[:, 0:6 * H])

                nc.gpsimd.dma_start(out=bv_row2[:], in_=bv_d[:].rearrange("(o d) -> o d", o=1))

            # V' gets ones in column 64 of each head slot
            nc.gpsimd.memset(vP[:], 1.0)

            # ---------------- projection emitters ----------------
            def emit_kq_block(which, c):
                """Project dout block c (heads 2c, 2c+1) of K or Q."""
                wsrc = wkT_d if which == "k" else wqT_d
                w = wkqp.tile([P, NT * P], BF16, tag="wblk")
                for t in range(NT):
                    nc.sync.dma_start(
                        out=w[:, t * P:(t + 1) * P],
                        in_=wsrc[t * P:(t + 1) * P, c * P:(c + 1) * P],
                    )
                dst = kT if which == "k" else qT
                bias_cols = bkcols if which == "k" else bqcols
                scale = 1.0 if which == "k" else 0.125
                for j in range(2):
                    ps = proj_ps.tile([P, 512], F32, tag="pps")
                    for kk in range(NT):
                        nc.tensor.matmul(
                            ps[:],
                            w[:, kk * P:(kk + 1) * P],
                            xT[:, kk * S + j * 512: kk * S + (j + 1) * 512],
                            start=(kk == 0),
                            stop=(kk == NT - 1),
                        )
                    nc.scalar.activation(
                        dst[:, c * S + j * 512: c * S + (j + 1) * 512], ps[:],
                        AF.Identity, bias=bias_cols[:, c:c + 1], scale=scale,
                    )

            _wv_tile = [None]

            def emit_v_half(half, sb_lo=0, sb_hi=NT):
                """Project V' for heads half*8 .. half*8+7 (dout cols 512*half..),
                seq-blocks sb_lo..sb_hi-1; the weight slice is DMA'd on the
                first chunk of each half and reused by later chunks."""
                if sb_lo == 0:
                    wv = wvp.tile([P, NT * 512], BF16, tag="wv")
                    for t in range(NT):
                        nc.sync.dma_start(
                            out=wv[:, t * 512:(t + 1) * 512],
                            in_=wvT_d[t * P:(t + 1) * P, half * 512:(half + 1) * 512],
                        )
                    _wv_tile[0] = wv
                else:
                    wv = _wv_tile[0]
                for sb in range(sb_lo, sb_hi):
                    ps = proj_ps.tile([P, 512], F32, tag="pps")
                    for kk in range(NT):
                        nc.tensor.matmul(
                            ps[:],
                            xT[:, kk * S + sb * P: kk * S + (sb + 1) * P],
                            wv[:, kk * 512:(kk + 1) * 512],
                            start=(kk == 0),
                            stop=False,
                        )
                    # + bv via a rank-1 accumulating matmul (ones column x bv row)
                    nc.tensor.matmul(
                        ps[:],
                        ones_row_bf[:],
                        bv_row2[:, half * 512:(half + 1) * 512],
                        start=False,
                        stop=True,
                    )
                    vslot = vP[
                        :, sb * H * 65 + half * 8 * 65: sb * H * 65 + (half * 8 + 8) * 65
                    ].rearrange("p (h e) -> p h e", h=8)[:, :, 0:HD]
                    nc.scalar.activation(
                        vslot, ps[:].rearrange("p (h e) -> p h e", h=8), AF.Copy,
                    )

            # ---------------- attention (with interleaved projections) --------
            def emit_ctx(h, pt, last=False):
                for qb in range(NT):
                    cps = cx_psp.tile([P, HD + 1], F32, tag="cps")
                    for kb in range(NT):
                        nc.tensor.matmul(
                            cps[:],
                            pt[:, kb * S + qb * P: kb * S + (qb + 1) * P],
                            vP[:, kb * H * 65 + h * 65: kb * H * 65 + (h + 1) * 65],
                            start=(kb == 0),
                            stop=(kb == NT - 1),
                        )
                    rc = rcp.tile([P, 1], F32, tag="rc")
                    nc.vector.reciprocal(rc[:], cps[:, HD:HD + 1])
                    nc.scalar.activation(
                        out_sb[:, qb * S + h * HD: qb * S + (h + 1) * HD],
                        cps[:, 0:HD], AF.Identity, bias=0.0, scale=rc[:],
                    )
                    if last:
                        nc.sync.dma_start(
                            out=out_d[qb * P:(qb + 1) * P, :],
                            in_=out_sb[:, qb * S:(qb + 1) * S],
                        )

            emit_kq_block("k", 0)
            emit_kq_block("q", 0)

            prev = None
            for h in range(H):
                off = (h % 2) * HD
                hc = h // 2
                pt = ptp.tile([P, NT * S], BF16, tag="pt")
                # k-tiles share one exp buffer so the custom-DVE ladder runs at
                # FD=4096 (amortizing per-op drain/dispatch overhead); head 0
                # uses 2+2+4 so the very first ladder op starts ~2 exp earlier.
                groups = (2, 2, 4) if h == 0 else (4, 4)
                kb0 = 0
                for gsz in groups:
                    ex = exp_pool.tile([P, 4 * S], BF16, tag="ex")
                    for kh in range(gsz):
                        kb = kb0 + kh
                        ps = sc_psp.tile([P, S], F32, tag="scps")
                        for j in range(2):
                            nc.tensor.matmul(
                                ps[:, j * 512:(j + 1) * 512],
                                kT[off:off + HD, hc * S + kb * P: hc * S + (kb + 1) * P],
                                qT[off:off + HD, hc * S + j * 512: hc * S + (j + 1) * 512],
                            )
                        nc.scalar.activation(
                            ex[:, kh * S:(kh + 1) * S], ps[:], AF.Exp,
                            bias=mcols[:, kb:kb + 1], scale=1.0,
                        )
                    kb = kb0
                    kb0 += gsz
                    ptk = pt[:, kb * S:(kb + gsz) * S]
                    t1 = lad.tile([P, 4 * S], BF16, tag="l1")
                    if _ABLATE == "ts":
                        # timing ablation: same op count/shapes, native 4x ops
                        nc.vector.tensor_scalar(
                            t1[:, 0:gsz * S], ex[:, 0:gsz * S], 1.0,
                            mprime[:, 0 * H + h: 0 * H + h + 1], OP.mult, OP.mult,
                        )
                        nc.vector.tensor_scalar(
                            ptk, t1[:, 0:gsz * S], 1.0,
                            mprime[:, 3 * H + h: 3 * H + h + 1], OP.mult, OP.mult,
                        )
                    else:
                        # pass A on (rel-3): m3 for rel<=3, m4/m5 for rel 4/5
                        nc.vector._custom_dve(
                            lut3, out=t1[:, 0:gsz * S],
                            in0=relm3[:, kb * S:(kb + gsz) * S], in1=ex[:, 0:gsz * S],
                            s0=mprime[:, 3 * H + h: 3 * H + h + 1],
                            s1=mprime[:, 4 * H + h: 4 * H + h + 1],
                            imm2=m5s[h],
                        )
                        # pass B on raw rel: m0/m3, m1/m3, m2/m3 for rel 0/1/2
                        nc.vector._custom_dve(
                            lut3, out=ptk, in0=rel0[:, kb * S:(kb + gsz) * S],
                            in1=t1[:, 0:gsz * S],
                            s0=mprime[:, 0 * H + h: 0 * H + h + 1],
                            s1=mprime[:, 1 * H + h: 1 * H + h + 1],
                            imm2=m2ns[h],
                        )

                # Interleave projection chunks so the Tensor engine always has
                # just-in-time work, spread as thin as dependencies allow:
                # V' halves are emitted in two sb-chunks straddling a head
                # boundary (the second chunk still precedes the ctx that needs
                # it, one head later); K/Q block c lands at head 2c-2, two
                # heads before scores(2c) consume it. ctx runs one head behind.
                if h == 0:
                    emit_v_half(0, 0, 5)
                else:
                    if h == 1:
                        emit_v_half(0, 5, 8)
                        emit_kq_block("k", 1)
                        emit_kq_block("q", 1)
                    elif h % 2 == 0 and 2 <= h <= 12:
                        emit_kq_block("k", h // 2 + 1)
                        emit_kq_block("q", h // 2 + 1)
                    if h == 8:
                        emit_v_half(1, 0, 5)
                    if h == 9:
                        emit_v_half(1, 5, 8)
                    emit_ctx(*prev)
                prev = (h, pt)

            emit_ctx(*prev, last=True)

    nc.compile()
    return nc


_PROGRAMS = {}


def _mtables(inputs):
    """Normalized multiplier table, host-side (O(NREL*H)=112 values of table
    prep, not per-element work). Rows 0..2 hold m'_r/m'_3 (pass-B entries),
    rows 3..5 hold m'_r (pass-A entries), with m'_r = exp(E[r]-E[6])."""
    remb = np.asarray(inputs["rel_emb"], dtype=np.float32)
    mp = np.exp(remb[0:6, :] - remb[6:7, :]).astype(np.float32)  # [6, H]
    out = mp.copy()
    out[0:3, :] = mp[0:3, :] / mp[3:4, :]
    return out


def _get_program(inputs, unroll=1):
    mt = _mtables(inputs)
    m5s = tuple(float(x) for x in mt[5])
    m2ns = tuple(float(x) for x in mt[2])
    key = (m5s, m2ns, _ABLATE, unroll)
    prog = _PROGRAMS.get(key)
    if prog is None:
        prog = _build_program((m5s, m2ns), unroll=unroll)
        _PROGRAMS[key] = prog
    return prog


def _make_in_maps(inputs):
    import ml_dtypes
    bf16 = ml_dtypes.bfloat16
    hidden = np.asarray(inputs["hidden_states"], dtype=np.float32)
    mask = np.asarray(inputs["attention_mask"], dtype=np.float32)
    relation = np.asarray(inputs["relation"], dtype=np.int32)
    wq = np.ascontiguousarray(np.asarray(inputs["Wq"], dtype=np.float32).T.astype(bf16))
    wk = np.ascontiguousarray(np.asarray(inputs["Wk"], dtype=np.float32).T.astype(bf16))
    wv = np.ascontiguousarray(np.asarray(inputs["Wv"], dtype=np.float32).T.astype(bf16))
    bq = np.asarray(inputs["bq"], dtype=np.float32)
    bk = np.asarray(inputs["bk"], dtype=np.float32)
    bv = np.asarray(inputs["bv"], dtype=np.float32)
    mtab = np.ascontiguousarray(_mtables(inputs).reshape(-1))  # [6*H]

    in_maps = []
    for b in range(N_CORES):
        in_maps.append({
            "xT": np.ascontiguousarray(hidden[b].T.astype(bf16)),
            "wqT": wq, "wkT": wk, "wvT": wv,
            "bq": bq, "bk": bk, "bv": bv,
            "relT": np.ascontiguousarray(relation[b].T.astype(np.int8)),
            "maskv": np.ascontiguousarray(mask[b, 0, 0, :]),
            "mtab": mtab,
        })
    return in_maps


LAST_EXEC_NS = None
LAST_RESULTS = None


def kernel(**inputs) -> np.ndarray:
    global LAST_EXEC_NS, LAST_RESULTS
    nc = _get_program(inputs)
    in_maps = _make_in_maps(inputs)
    trace = os.environ.get("KERNEL_TRACE", "0") == "1"
    res = run_bass_kernel_spmd(nc, in_maps, list(range(N_CORES)), trace=trace)
    LAST_EXEC_NS = res.exec_time_ns
    LAST_RESULTS = res
    out = np.stack([res.results[b]["out"] for b in range(N_CORES)], axis=0)
    return out.astype(np.float32)


# -------- timing helper: device-resident repeated dispatch --------
def make_bench_fn(inputs, unroll=1):
    """Returns run(reps) -> min wall seconds over reps dispatches of the
    unroll-times-unrolled program (device-resident inputs)."""
    import jax
    from jax.sharding import Mesh, PartitionSpec, NamedSharding
    from jax.experimental.shard_map import shard_map
    from concourse import bass2jax
    import concourse.mybir as mb

    nc = _get_program(inputs, unroll=unroll)
    in_maps = _make_in_maps(inputs)
    bass2jax.install_neuronx_cc_hook()

    part_name = nc.partition_id_tensor.name if nc.partition_id_tensor else None
    in_names, out_names, out_avals, zero_outs = [], [], [], []
    for alloc in nc.m.functions[0].allocations:
        if not isinstance(alloc, mb.MemoryLocationSet):
            continue
        name = alloc.memorylocations[0].name
        if alloc.kind == "ExternalInput":
            if name != part_name:
                in_names.append(name)
        elif alloc.kind == "ExternalOutput":
            out_names.append(name)
            shape = tuple(alloc.tensor_shape)
            dtype = mb.dt.np(alloc.dtype)
            out_avals.append(jax.core.ShapedArray(shape, dtype))
            zero_outs.append(np.zeros(shape, dtype))
    n_params = len(in_names)
    all_names = in_names + out_names
    if part_name is not None:
        all_names.append(part_name)

    def _body(*args):
        operands = list(args)
        if part_name is not None:
            operands.append(bass2jax.partition_id_tensor())
        outs = bass2jax._bass_exec_p.bind(
            *operands,
            out_avals=tuple(out_avals),
            in_names=tuple(all_names),
            out_names=tuple(out_names),
            lowering_input_output_aliases=(),
            sim_require_finite=True,
            sim_require_nnan=True,
            nc=nc,
        )
        return tuple(outs)

    devices = jax.devices()[:N_CORES]
    mesh = Mesh(np.asarray(devices), ("core",))
    n_all = n_params + len(out_names)
    sharded = jax.jit(
        shard_map(
            _body, mesh=mesh,
            in_specs=(PartitionSpec("core"),) * n_all,
            out_specs=(PartitionSpec("core"),) * len(out_names),
            check_rep=False,
        ),
        keep_unused=True,
    )
    sh = NamedSharding(mesh, PartitionSpec("core"))
    concat_in = [
        jax.device_put(
            np.concatenate([np.asarray(in_maps[c][nm]) for c in range(N_CORES)], axis=0), sh
        )
        for nm in in_names
    ]
    concat_zeros = [
        jax.device_put(np.zeros((N_CORES * z.shape[0], *z.shape[1:]), z.dtype), sh)
        for z in zero_outs
    ]

    # warmup + compile
    out = sharded(*concat_in, *concat_zeros)
    jax.block_until_ready(out)

    import time

    def run(reps=1):
        best = float("inf")
        for _ in range(reps):
            t0 = time.perf_counter()
            outs = sharded(*concat_in, *concat_zeros)
            jax.block_until_ready(outs)
            best = min(best, time.perf_counter() - t0)
        return best

    def batch(M):
        """Wall seconds for M back-to-back (non-blocking) dispatches."""
        t0 = time.perf_counter()
        outs = None
        for _ in range(M):
            outs = sharded(*concat_in, *concat_zeros)
        jax.block_until_ready(outs)
        return time.perf_counter() - t0

    def get_out():
        outs = sharded(*concat_in, *concat_zeros)
        o = np.asarray(outs[0]).reshape(N_CORES, *out_avals[0].shape)
        return o

    run.get_out = get_out
    run.batch = batch
    return run


# -------- simulation helper (single core) for test.py --------
def run_sim_core0(inputs):
    from concourse.bass_interp import CoreSim

    nc = _get_program(inputs)
    in_maps = _make_in_maps(inputs)
    sim = CoreSim(nc, trace=False)
    for k, v in in_maps[0].items():
        sim.tensor(k)[:] = v
    sim.simulate(check_with_hw=False)
    return np.array(sim.tensor("out"))
